# revision 1
# baseline (speedup 1.0000x reference)
"""Multi-head attention (B=8, S=1024, E=768, H=12, D=64) on 8 TRN2 NeuronCores.

Sharding: data-parallel over batch. Core i computes batch element i end to end;
weights are replicated. No collectives.

Per-core dataflow (all matmuls float32r unless noted; Q/K in bf16):
  1. x [S,E] -> PE-transpose -> xT [E,S] (f32r)
  2. QT/KT = w_qkv.T @ xT (bf16); V -> V_pad [S, H*(D+1)] with a ones column
  3. attention per head pair, per q-half (512 queries), packed K=64 score
     matmuls via tile_position; exp on ACT in [128,512] chunks over four
     1-bank PSUM score slots; PV accumulates out^T[d,q] plus a denominator
     row; normalize = PE broadcast + fast reciprocal
  4. y = attnT.T @ w_out + b_out

The V and Q/K projection work is interleaved into the attention loop as PE
"filler" chunks: attention alone leaves the PE ~55% idle waiting on ACT exp,
which also parks the PE clock at 1.2 GHz (HAM never sees a dense-activity
window). Interleaving keeps the PE dense and the clock at 2.4 GHz.

PSUM budget (8 banks): scores 4 x 1 + pv 2 x 1 + mm 2 x 1.
"""

import numpy as np

import concourse.bass as bass
import concourse.bacc as bacc
import concourse.tile as tile
from concourse import mybir
from concourse.bass_utils import run_bass_kernel_spmd
from concourse.bass_interp import get_hw_module
from concourse.masks import make_identity

F32 = mybir.dt.float32
F32R = mybir.dt.float32r
BF16 = mybir.dt.bfloat16
U32 = mybir.dt.uint32

B, S, E = 8, 1024, 768
H, D = 12, 64
F = 3 * E                  # 2304
NCORES = 8
NPAIR = H // 2             # 6 head pairs
NKC = S // 128             # 8 key chunks
NST = S // 128             # 8 sequence tiles
NE = E // 128              # 6 embedding chunks
DP = D + 1                 # 65: head dim + ones column

PV_DT = F32R               # dtype of exp(S^T) and V_pad
QK_DT = BF16               # dtype of Q^T / K^T


def _build():
    nc = bacc.Bacc("TRN2", target_bir_lowering=False, debug=False,
                   num_devices=NCORES)

    x_d = nc.dram_tensor("x", [S, E], F32, kind="ExternalInput").ap()
    wqkv_d = nc.dram_tensor("w_qkv", [E, F], F32, kind="ExternalInput").ap()
    wout_d = nc.dram_tensor("w_out", [E, E], F32, kind="ExternalInput").ap()
    bout_d = nc.dram_tensor("b_out", [E], F32, kind="ExternalInput").ap()
    y_d = nc.dram_tensor("y", [S, E], F32, kind="ExternalOutput").ap()

    with tile.TileContext(nc) as tc:
        _emit(nc, tc, x_d, wqkv_d, wout_d, bout_d, y_d)

    nc.compile()
    nc.m = get_hw_module(nc.m)
    return nc


def _emit(nc, tc, x_d, wqkv_d, wout_d, bout_d, y_d):
    from contextlib import ExitStack
    ctx = ExitStack()
    with ctx:
        singles = ctx.enter_context(tc.tile_pool(name="singles", bufs=1))
        sb = ctx.enter_context(tc.tile_pool(name="sb", bufs=1))
        ps = ctx.enter_context(tc.tile_pool(name="ps", bufs=1, space="PSUM"))
        expst_pool = ctx.enter_context(tc.tile_pool(name="expst", bufs=3))
        bcast_pool = ctx.enter_context(tc.tile_pool(name="bcast", bufs=2))
        rc_pool = ctx.enter_context(tc.tile_pool(name="rc", bufs=4))
        ypool = ctx.enter_context(tc.tile_pool(name="ypool", bufs=2))

        # ---- constants ----
        identity = singles.tile([128, 128], F32)
        make_identity(nc, identity)
        bias_bc = singles.tile([128, E], F32)
        ones_row = singles.tile([1, 64], F32R)
        nc.gpsimd.memset(ones_row.bitcast(U32), 0x3F800000)

        wq_pool = tc.alloc_tile_pool(name="wq_pool", bufs=1)
        x_pool = tc.alloc_tile_pool(name="x_pool", bufs=1)

        # ---- weights + x (DMA priority: x, then Q/K columns, then V) ----
        wq = [wq_pool.tile([128, F], F32R, name=f"wqkv_{ei}")
              for ei in range(NE)]

        def dma_w_cols(c0, c1):
            for ei in range(NE):
                nc.sync.dma_start(
                    out=wq[ei][:, c0:c1],
                    in_=wqkv_d[ei * 128:(ei + 1) * 128, c0:c1].bitcast(F32R))
        # ---- transpose x -> xT [E, S] (f32r) via PE, half of x at a time ----
        xT = [wq_pool.tile([128, S], F32R, name=f"xT_{ei}")
              for ei in range(NE)]
        for half in range(2):
            x_sb = []
            for k in range(4):
                st = half * 4 + k
                x_t = x_pool.tile([128, E], F32, tag="x", bufs=4,
                                  name=f"x_{st}")
                nc.sync.dma_start(out=x_t, in_=x_d[st * 128:(st + 1) * 128, :])
                x_sb.append(x_t)
            if half == 0:
                dma_w_cols(0, 128)                    # Q pair 0
                dma_w_cols(E, E + 128)                # K pair 0
            else:
                dma_w_cols(2 * E, F)                  # V columns (PV needs
                dma_w_cols(128, E)                    # them from kc=1 on)
                dma_w_cols(E + 128, 2 * E)            # Q/K pairs 1-5
            for ei in range(NE):
                ps_xt = ps.tile([128, 512], F32, tag="mm", bufs=2,
                                name=f"psxt_{ei}_{half}")
                for k in range(4):
                    nc.tensor.transpose(
                        ps_xt[:, k * 128:(k + 1) * 128],
                        x_sb[k][:, ei * 128:(ei + 1) * 128],
                        identity)
                # alternate DVE/ACT so the psum slot pair is drained at 2x
                # rate (ACT is idle during the prelude)
                dst = xT[ei][:, half * 512:(half + 1) * 512]
                if ei % 2 == 0:
                    nc.vector.tensor_copy(dst, ps_xt)
                else:
                    nc.scalar.copy(dst, ps_xt)
        x_pool.release()

        # ---- projection chunk emitters (used as PE fillers) ----
        v_pad = [sb.tile([128, H * DP], PV_DT, name=f"vpad_{st}")
                 for st in range(NST)]
        for st in range(NST):
            nc.gpsimd.memset(v_pad[st].bitcast(U32), 0x3F800000)
        qkT = [sb.tile([128, S], QK_DT, name=f"qkT_{ft}")
               for ft in range(2 * NE)]

        def emit_v_chunk(st, c0, cn):
            ps_v = ps.tile([128, 512], F32, tag="mm", bufs=2,
                           name=f"psv_{st}_{c0}")
            for ei in range(NE):
                nc.tensor.matmul(
                    ps_v[:, 0:cn],
                    xT[ei][:, st * 128:(st + 1) * 128],
                    wq[ei][:, 2 * E + c0:2 * E + c0 + cn],
                    start=(ei == 0), stop=(ei == NE - 1))
            vp3 = v_pad[st].rearrange("p (h c) -> p h c", c=DP)
            h0 = c0 // D
            nc.vector.tensor_copy(
                vp3[:, h0:h0 + cn // D, 0:D],
                ps_v[:, 0:cn].rearrange("p (h d) -> p h d", d=D))

        def emit_qkt_chunk(ft, sc, on_act=False):
            ps_q = ps.tile([128, 512], F32, tag="mm", bufs=2,
                           name=f"psq_{ft}_{sc}")
            for ei in range(NE):
                nc.tensor.matmul(
                    ps_q,
                    wq[ei][:, ft * 128:(ft + 1) * 128],
                    xT[ei][:, sc * 512:(sc + 1) * 512],
                    start=(ei == 0), stop=(ei == NE - 1))
            dst = qkT[ft][:, sc * 512:(sc + 1) * 512]
            if on_act:
                nc.scalar.copy(dst, ps_q)
            else:
                nc.vector.tensor_copy(dst, ps_q)

        # prelude: only the q-lower-half of pair-0's Q/K gates the first
        # scores matmul (score chunks kc<4 read kT cols < 512)
        emit_qkt_chunk(0, 0, on_act=True)
        emit_qkt_chunk(NE, 0, on_act=True)

        # filler schedule: (j, qh, kc) -> list of emit thunks
        filler_schedule = {}

        def sched(j, qh, kc, thunk):
            filler_schedule.setdefault((j, qh, kc), []).append(thunk)

        sched(0, 0, 0, lambda: emit_qkt_chunk(0, 1))
        sched(0, 0, 0, lambda: emit_qkt_chunk(NE, 1))
        sched(0, 0, 1, lambda: emit_v_chunk(0, 0, 512))
        sched(0, 0, 1, lambda: emit_v_chunk(0, 512, 256))
        for st in range(1, NST):
            kc = min(1 + (st + 1) // 2, 6)
            sched(0, 0, kc, lambda st=st: emit_v_chunk(st, 0, 512))
            sched(0, 0, kc, lambda st=st: emit_v_chunk(st, 512, 256))
        for j in range(1, NPAIR):
            for i, (ft, sc) in enumerate(
                    [(j, 0), (NE + j, 0), (j, 1), (NE + j, 1)]):
                sched(j - 1, i // 2, 3 + 3 * (i % 2),
                      lambda ft=ft, sc=sc: emit_qkt_chunk(ft, sc))

        def pop_filler(j, qh, kc):
            for thunk in filler_schedule.pop((j, qh, kc), ()):
                thunk()

        # ---- attention: per pair, per q-half, fillers interleaved ----
        # The normalize tail of each (j, qh) is split: the cheap denominator
        # copies run immediately, but the PE-broadcast/reciprocal/multiply
        # chain is deferred into the next (j, qh)'s kc loop so the PE and ACT
        # never sit behind it.
        attnT = []
        pending_finish = [None]
        pending_norm = [None]

        def emit_finish_tail():
            if pending_finish[0] is not None:
                pending_finish[0]()
                pending_finish[0] = None

        def emit_norm_tail():
            emit_finish_tail()
            if pending_norm[0] is not None:
                pending_norm[0]()
                pending_norm[0] = None

        for j in range(NPAIR):
            qT = qkT[j]
            kT = qkT[NE + j]
            at_t = sb.tile([128, S], F32R, name=f"attnT_{j}")
            for qh in range(2):
                q0 = qh * 512
                ps_pv = [ps.tile([DP, 512], F32, tag="pv", bufs=2,
                                 name=f"pspv_{j}_{qh}_{hh}")
                         for hh in range(2)]
                expst_tiles = {}

                def emit_pv(kc, j=j, qh=qh, ps_pv=ps_pv,
                            expst_tiles=expst_tiles):
                    expst = expst_tiles.pop(kc)
                    for hh in range(2):
                        nc.tensor.matmul(
                            ps_pv[hh],
                            v_pad[kc][:, (2 * j + hh) * DP:
                                       (2 * j + hh + 1) * DP],
                            expst[:, hh * 512:(hh + 1) * 512],
                            start=(kc == 0), stop=(kc == NKC - 1))

                for kc in range(NKC):
                    expst = expst_pool.tile([128, 1024], PV_DT, tag="expst",
                                            name=f"expst_{j}_{qh}_{kc}")
                    expst_tiles[kc] = expst
                    # both heads' scores into one 2-bank tile -> single
                    # [128,1024] exp (ACT per-op overhead paid once)
                    ps_s = ps.tile([128, 1024], F32, tag="scores", bufs=2,
                                   name=f"pss_{j}_{qh}_{kc}")
                    for hh in range(2):
                        nc.tensor.matmul(
                            ps_s[:, hh * 512:(hh + 1) * 512],
                            kT[hh * 64:(hh + 1) * 64,
                               kc * 128:(kc + 1) * 128],
                            qT[hh * 64:(hh + 1) * 64, q0:q0 + 512],
                            start=True, stop=True,
                            tile_position=(hh * 64, 0))
                    nc.scalar.activation(
                        out=expst, in_=ps_s,
                        func=mybir.ActivationFunctionType.Exp,
                        scale=0.125)
                    if kc == 0:
                        emit_finish_tail()
                    elif kc == 1:
                        emit_norm_tail()
                    pop_filler(j, qh, kc)
                    if kc > 0:
                        emit_pv(kc - 1)

                rcs = []

                def finish_tail(j=j, qh=qh, ps_pv=ps_pv, rcs=rcs,
                                emit_pv=emit_pv):
                    emit_pv(NKC - 1)
                    for hh in range(2):
                        rc_t = rc_pool.tile([1, 512], F32R, tag="rc",
                                            name=f"rc_{j}_{qh}_{hh}")
                        nc.vector.tensor_copy(rc_t, ps_pv[hh][D:DP, :])
                        rcs.append(rc_t)

                def norm_tail(j=j, qh=qh, q0=q0, ps_pv=ps_pv, rcs=rcs,
                              at_t=at_t):
                    bc_sb = bcast_pool.tile([64, 1024], F32, tag="bc",
                                            name=f"bc_{j}_{qh}")
                    bc_ps = ps.tile([64, 1024], F32, tag="scores", bufs=2,
                                    name=f"bcps_{j}_{qh}")
                    for hh in range(2):
                        nc.tensor.matmul(bc_ps[:, hh * 512:(hh + 1) * 512],
                                         ones_row, rcs[hh],
                                         start=True, stop=True)
                    nc.vector.reciprocal_approx_fast(out=bc_sb, in_=bc_ps)
                    for hh in range(2):
                        nc.vector.tensor_mul(
                            at_t[hh * 64:(hh + 1) * 64, q0:q0 + 512],
                            ps_pv[hh][0:D, :],
                            bc_sb[:, hh * 512:(hh + 1) * 512])

                pending_finish[0] = finish_tail
                pending_norm[0] = norm_tail
            attnT.append(at_t)
        emit_norm_tail()
        for key in sorted(filler_schedule):
            for thunk in filler_schedule[key]:
                thunk()
        filler_schedule.clear()
        wq_pool.release()

        # ---- output projection + bias ----
        wpool = ctx.enter_context(tc.tile_pool(name="wpool", bufs=1))
        wo = []
        for ei in range(NE):
            wo_t = wpool.tile([128, E], F32R, name=f"wout_{ei}")
            nc.sync.dma_start(out=wo_t,
                              in_=wout_d[ei * 128:(ei + 1) * 128, :].bitcast(F32R))
            wo.append(wo_t)
        nc.sync.dma_start(
            out=bias_bc,
            in_=bass.AP(tensor=bout_d.tensor, offset=bout_d.offset,
                        ap=[[0, 128]] + list(bout_d.ap)))

        for st in range(NST):
            y_t = ypool.tile([128, E], F32, tag="y", name=f"y_{st}")
            for (c0, cn) in ((0, 512), (512, 256)):
                ps_y = ps.tile([128, 512], F32, tag="mm", bufs=2,
                               name=f"psy_{st}_{c0}")
                for ej in range(NE):
                    nc.tensor.matmul(
                        ps_y[:, 0:cn],
                        attnT[ej][:, st * 128:(st + 1) * 128],
                        wo[ej][:, c0:c0 + cn],
                        start=(ej == 0), stop=(ej == NE - 1))
                nc.vector.tensor_add(y_t[:, c0:c0 + cn], ps_y[:, 0:cn],
                                     bias_bc[:, c0:c0 + cn])
            nc.sync.dma_start(out=y_d[st * 128:(st + 1) * 128, :], in_=y_t)


_NC_CACHE = None


def _get_nc():
    global _NC_CACHE
    if _NC_CACHE is None:
        _NC_CACHE = _build()
    return _NC_CACHE


def kernel(x, w_qkv, w_out, b_out, _trace=False, **_run_kwargs):
    """Full-input MHA: x [8,1024,768] f32 -> y [8,1024,768] f32."""
    nc = _get_nc()
    x = np.ascontiguousarray(np.asarray(x, dtype=np.float32))
    w_qkv = np.ascontiguousarray(np.asarray(w_qkv, dtype=np.float32))
    w_out = np.ascontiguousarray(np.asarray(w_out, dtype=np.float32))
    b_out = np.ascontiguousarray(np.asarray(b_out, dtype=np.float32))
    in_maps = [
        {"x": x[i], "w_qkv": w_qkv, "w_out": w_out, "b_out": b_out}
        for i in range(NCORES)
    ]
    res = run_bass_kernel_spmd(nc, in_maps, core_ids=list(range(NCORES)),
                               trace=_trace, **_run_kwargs)
    y = np.stack([res.results[i]["y"] for i in range(NCORES)], axis=0)
    if _trace:
        return y, res
    return y



# revision 8
# speedup vs baseline: 1.0769x; 1.0769x over previous
"""Multi-head attention (B=8, S=1024, E=768, H=12, D=64) on 8 TRN2 NeuronCores.

Sharding: data-parallel over batch. Core i computes batch element i end to end;
weights are replicated. No collectives.

All matmul operands are bf16: on TRN2 the PE streams moving data at 2B/partition
/cycle, so f32r moving data runs ~1.7x slower than bf16 (confirmed by trace:
f32r matmuls 339-361ns per 512 cols vs bf16's 213ns). Weights are DMA'd as f32
into staging tiles and cast to bf16 on ACT/DVE, overlapped with the x-transpose
prelude and the attention loop (cast thunks ride the PE filler schedule).

Per-core dataflow:
  1. x [S,E] -> bf16 -> PE-transpose -> xT [E,S] bf16
  2. QT/KT = w_qkv.T @ xT (bf16); V -> V_pad [S, H*(D+1)] with a ones column
  3. attention per head pair, per q-half (512 queries), packed K=64 score
     matmuls via tile_position; exp on ACT in [128,1024] chunks (bf16 out);
     PV accumulates out^T[d,q] plus a denominator row; normalize = PE
     broadcast + fast reciprocal
  4. y = attnT.T @ w_out + b_out

The V and Q/K projection work is interleaved into the attention loop as PE
"filler" chunks to keep the PE dense (p-state at 2.4 GHz) while ACT runs exp.

PSUM budget (8 banks): scores 4 + pv 2 + mm 2.
"""

import numpy as np

import concourse.bass as bass
import concourse.bacc as bacc
import concourse.tile as tile
from concourse import mybir
from concourse.bass_utils import run_bass_kernel_spmd
from concourse.bass_interp import get_hw_module
from concourse.masks import make_identity

F32 = mybir.dt.float32
F32R = mybir.dt.float32r
BF16 = mybir.dt.bfloat16
U32 = mybir.dt.uint32

B, S, E = 8, 1024, 768
H, D = 12, 64
F = 3 * E                  # 2304
NCORES = 8
NPAIR = H // 2             # 6 head pairs
NKC = S // 128             # 8 key chunks
NST = S // 128             # 8 sequence tiles
NE = E // 128              # 6 embedding chunks
DP = D + 1                 # 65: head dim + ones column

BF16_ONES = 0x3F803F80     # two packed bf16 1.0


def _build():
    nc = bacc.Bacc("TRN2", target_bir_lowering=False, debug=False,
                   num_devices=NCORES)

    x_d = nc.dram_tensor("x", [S, E], F32, kind="ExternalInput").ap()
    wqkv_d = nc.dram_tensor("w_qkv", [E, F], F32, kind="ExternalInput").ap()
    wout_d = nc.dram_tensor("w_out", [E, E], F32, kind="ExternalInput").ap()
    bout_d = nc.dram_tensor("b_out", [E], F32, kind="ExternalInput").ap()
    y_d = nc.dram_tensor("y", [S, E], F32, kind="ExternalOutput").ap()

    with tile.TileContext(nc) as tc:
        _emit(nc, tc, x_d, wqkv_d, wout_d, bout_d, y_d)

    nc.compile()
    nc.m = get_hw_module(nc.m)
    return nc


def _emit(nc, tc, x_d, wqkv_d, wout_d, bout_d, y_d):
    from contextlib import ExitStack
    ctx = ExitStack()
    with ctx:
        singles = ctx.enter_context(tc.tile_pool(name="singles", bufs=1))
        sb = ctx.enter_context(tc.tile_pool(name="sb", bufs=1))
        ps = ctx.enter_context(tc.tile_pool(name="ps", bufs=1, space="PSUM"))
        expst_pool = ctx.enter_context(tc.tile_pool(name="expst", bufs=3))
        bcast_pool = ctx.enter_context(tc.tile_pool(name="bcast", bufs=2))
        rc_pool = ctx.enter_context(tc.tile_pool(name="rc", bufs=4))
        ypool = ctx.enter_context(tc.tile_pool(name="ypool", bufs=2))

        # ---- constants ----
        identity = singles.tile([128, 128], BF16)
        make_identity(nc, identity)
        bias_bc = singles.tile([128, E], F32)
        ones_row = singles.tile([1, 64], BF16)
        nc.gpsimd.memset(ones_row.bitcast(U32), BF16_ONES)

        wpool = ctx.enter_context(tc.tile_pool(name="wpool", bufs=1))
        wq_pool = tc.alloc_tile_pool(name="wq_pool", bufs=1)
        wst_pool = tc.alloc_tile_pool(name="wst_pool", bufs=1)
        x_pool = tc.alloc_tile_pool(name="x_pool", bufs=1)

        # bf16 weights for QKV projection: wq[ei] holds rows [128*ei, 128*ei+128)
        wq = [wq_pool.tile([128, F], BF16, name=f"wqkv_{ei}")
              for ei in range(NE)]

        # f32 staging for weight chunks (DMA f32 -> cast bf16).
        # Column groups per ei: Q0 [0,128), K0 [E,E+128), V [2E,3E),
        # Qrest [128,E), Krest [E+128,2E).
        def dma_w_group(ei, c0, cn, tag, bufs):
            st_t = wst_pool.tile([128, cn], F32, tag=tag, bufs=bufs,
                                 name=f"wst_{ei}_{c0}")
            nc.sync.dma_start(out=st_t,
                              in_=wqkv_d[ei * 128:(ei + 1) * 128, c0:c0 + cn])
            return st_t

        def cast_w_group(ei, c0, st_t, on_act):
            dst = wq[ei][:, c0:c0 + st_t.shape[1]]
            if on_act:
                nc.scalar.copy(dst, st_t)
            else:
                nc.vector.tensor_copy(dst, st_t)

        # ---- x -> bf16 -> PE transpose -> xT [E, S] bf16 ----
        xT = [wq_pool.tile([128, S], BF16, name=f"xT_{ei}")
              for ei in range(NE)]
        w_stage = {}
        for half in range(2):
            xb_sb = []
            for k in range(4):
                st = half * 4 + k
                x_t = x_pool.tile([128, E], F32, tag="x", bufs=4,
                                  name=f"x_{st}")
                nc.sync.dma_start(out=x_t, in_=x_d[st * 128:(st + 1) * 128, :])
                xb = x_pool.tile([128, E], BF16, tag="xb", bufs=4,
                                 name=f"xb_{st}")
                if k % 2 == 0:
                    nc.vector.tensor_copy(xb, x_t)
                else:
                    nc.scalar.copy(xb, x_t)
                xb_sb.append(xb)
            if half == 0:
                for ei in range(NE):
                    w_stage[(ei, 0)] = dma_w_group(ei, 0, 128, "wsts", 12)
                    w_stage[(ei, E)] = dma_w_group(ei, E, 128, "wsts", 12)
            else:
                for ei in range(NE):
                    w_stage[(ei, 2 * E)] = dma_w_group(ei, 2 * E, E,
                                                       "wstb", 9)
                for ei in range(NE):
                    w_stage[(ei, 128)] = dma_w_group(ei, 128, E - 128,
                                                     "wstb", 9)
                for ei in range(NE):
                    w_stage[(ei, E + 128)] = dma_w_group(ei, E + 128, E - 128,
                                                         "wstb", 9)
            for ei in range(NE):
                ps_xt = ps.tile([128, 512], BF16, tag="mm", bufs=2,
                                name=f"psxt_{ei}_{half}")
                for k in range(4):
                    nc.tensor.transpose(
                        ps_xt[:, k * 128:(k + 1) * 128],
                        xb_sb[k][:, ei * 128:(ei + 1) * 128],
                        identity)
                dst = xT[ei][:, half * 512:(half + 1) * 512]
                if ei % 2 == 0:
                    nc.vector.tensor_copy(dst, ps_xt)
                else:
                    nc.scalar.copy(dst, ps_xt)
            # after the half-0 transposes: cast Q0/K0 groups (needed by the
            # prelude qkt chunks below)
            if half == 0:
                for ei in range(NE):
                    cast_w_group(ei, 0, w_stage[(ei, 0)], on_act=(ei % 2 == 0))
                    cast_w_group(ei, E, w_stage[(ei, E)], on_act=(ei % 2 == 1))
        x_pool.release()

        # ---- projection chunk emitters (used as PE fillers) ----
        v_pad = [sb.tile([128, H * DP], BF16, name=f"vpad_{st}")
                 for st in range(NST)]
        for st in range(NST):
            nc.gpsimd.memset(v_pad[st].bitcast(U32), BF16_ONES)
        qkT = [sb.tile([128, S], BF16, name=f"qkT_{ft}")
               for ft in range(2 * NE)]

        def emit_v_chunk(st, c0, cn):
            ps_v = ps.tile([128, 512], F32, tag="mm", bufs=2,
                           name=f"psv_{st}_{c0}")
            for ei in range(NE):
                nc.tensor.matmul(
                    ps_v[:, 0:cn],
                    xT[ei][:, st * 128:(st + 1) * 128],
                    wq[ei][:, 2 * E + c0:2 * E + c0 + cn],
                    start=(ei == 0), stop=(ei == NE - 1))
            vp3 = v_pad[st].rearrange("p (h c) -> p h c", c=DP)
            h0 = c0 // D
            nc.vector.tensor_copy(
                vp3[:, h0:h0 + cn // D, 0:D],
                ps_v[:, 0:cn].rearrange("p (h d) -> p h d", d=D))

        def emit_qkt_chunk(ft, sc, on_act=False):
            ps_q = ps.tile([128, 512], F32, tag="mm", bufs=2,
                           name=f"psq_{ft}_{sc}")
            for ei in range(NE):
                nc.tensor.matmul(
                    ps_q,
                    wq[ei][:, ft * 128:(ft + 1) * 128],
                    xT[ei][:, sc * 512:(sc + 1) * 512],
                    start=(ei == 0), stop=(ei == NE - 1))
            dst = qkT[ft][:, sc * 512:(sc + 1) * 512]
            if on_act:
                nc.scalar.copy(dst, ps_q)
            else:
                nc.vector.tensor_copy(dst, ps_q)

        # prelude: only the q-lower-half of pair-0's Q/K gates the first
        # scores matmul (score chunks kc<4 read kT cols < 512)
        emit_qkt_chunk(0, 0, on_act=True)
        emit_qkt_chunk(NE, 0, on_act=True)

        # V casts: needed before emit_v_chunk(0) at filler (0,0,1).
        # DVE-only: the ACT exp chain must never block on weight DMA.
        for ei in range(NE):
            cast_w_group(ei, 2 * E, w_stage[(ei, 2 * E)], on_act=False)

        # ---- w_out staging (DMA early, cast via fillers) ----
        wo = [wpool.tile([128, E], BF16, name=f"wout_{ei}")
              for ei in range(NE)]
        wo_stage = {}
        for ei in range(NE):
            st_t = wst_pool.tile([128, E], F32, tag="wstb", bufs=9,
                                 name=f"wost_{ei}")
            nc.sync.dma_start(
                out=st_t, in_=wout_d[ei * 128:(ei + 1) * 128, :])
            wo_stage[ei] = st_t
        nc.sync.dma_start(
            out=bias_bc,
            in_=bass.AP(tensor=bout_d.tensor, offset=bout_d.offset,
                        ap=[[0, 128]] + list(bout_d.ap)))

        # filler schedule: (j, qh, kc) -> list of emit thunks
        filler_schedule = {}

        def sched(j, qh, kc, thunk):
            filler_schedule.setdefault((j, qh, kc), []).append(thunk)

        # Qrest casts (cols [128, E)): needed by qkt fillers for pair 1 at
        # (0,0,3).  Krest (cols [E+128, 2E)): needed at (0,0,6).  All weight
        # casts run on DVE so the ACT exp chain never blocks on weight DMA.
        for ei in range(NE):
            sched(0, 0, ei // 2,
                  lambda ei=ei: cast_w_group(ei, 128, w_stage[(ei, 128)],
                                             on_act=False))
        for ei in range(NE):
            sched(0, 0, 3 + ei // 2,
                  lambda ei=ei: cast_w_group(ei, E + 128,
                                             w_stage[(ei, E + 128)],
                                             on_act=False))
        # w_out casts: needed only by the output projection at the end
        for ei in range(NE):
            sched(1, ei % 2, 2 + 2 * (ei // 2),
                  lambda ei=ei: nc.vector.tensor_copy(wo[ei], wo_stage[ei]))

        sched(0, 0, 0, lambda: emit_qkt_chunk(0, 1))
        sched(0, 0, 0, lambda: emit_qkt_chunk(NE, 1))
        sched(0, 0, 1, lambda: emit_v_chunk(0, 0, 512))
        sched(0, 0, 1, lambda: emit_v_chunk(0, 512, 256))
        for st in range(1, NST):
            kc = min(1 + (st + 1) // 2, 6)
            sched(0, 0, kc, lambda st=st: emit_v_chunk(st, 0, 512))
            sched(0, 0, kc, lambda st=st: emit_v_chunk(st, 512, 256))
        for j in range(1, NPAIR):
            for i, (ft, sc) in enumerate(
                    [(j, 0), (NE + j, 0), (j, 1), (NE + j, 1)]):
                sched(j - 1, i // 2, 3 + 3 * (i % 2),
                      lambda ft=ft, sc=sc: emit_qkt_chunk(ft, sc))

        def pop_filler(j, qh, kc):
            for thunk in filler_schedule.pop((j, qh, kc), ()):
                thunk()

        # ---- attention: per pair, per q-half, fillers interleaved ----
        # The normalize tail of each (j, qh) is split: the cheap denominator
        # copies run immediately, but the PE-broadcast/reciprocal/multiply
        # chain is deferred into the next (j, qh)'s kc loop so the PE and ACT
        # never sit behind it.
        attnT = []
        pending_finish = [None]
        pending_norm = [None]

        def emit_finish_tail():
            if pending_finish[0] is not None:
                pending_finish[0]()
                pending_finish[0] = None

        def emit_norm_tail():
            emit_finish_tail()
            if pending_norm[0] is not None:
                pending_norm[0]()
                pending_norm[0] = None

        for j in range(NPAIR):
            qT = qkT[j]
            kT = qkT[NE + j]
            at_t = sb.tile([128, S], BF16, name=f"attnT_{j}")
            for qh in range(2):
                q0 = qh * 512
                ps_pv = [ps.tile([DP, 512], F32, tag="pv", bufs=2,
                                 name=f"pspv_{j}_{qh}_{hh}")
                         for hh in range(2)]
                expst_tiles = {}

                def emit_pv(kc, j=j, qh=qh, ps_pv=ps_pv,
                            expst_tiles=expst_tiles):
                    expst = expst_tiles.pop(kc)
                    for hh in range(2):
                        nc.tensor.matmul(
                            ps_pv[hh],
                            v_pad[kc][:, (2 * j + hh) * DP:
                                       (2 * j + hh + 1) * DP],
                            expst[:, hh * 512:(hh + 1) * 512],
                            start=(kc == 0), stop=(kc == NKC - 1))

                for kc in range(NKC):
                    expst = expst_pool.tile([128, 1024], BF16, tag="expst",
                                            name=f"expst_{j}_{qh}_{kc}")
                    expst_tiles[kc] = expst
                    # both heads' scores into one 2-bank tile -> single
                    # [128,1024] exp (ACT per-op overhead paid once)
                    ps_s = ps.tile([128, 1024], F32, tag="scores", bufs=2,
                                   name=f"pss_{j}_{qh}_{kc}")
                    for hh in range(2):
                        nc.tensor.matmul(
                            ps_s[:, hh * 512:(hh + 1) * 512],
                            kT[hh * 64:(hh + 1) * 64,
                               kc * 128:(kc + 1) * 128],
                            qT[hh * 64:(hh + 1) * 64, q0:q0 + 512],
                            start=True, stop=True,
                            tile_position=(hh * 64, 0))
                    nc.scalar.activation(
                        out=expst, in_=ps_s,
                        func=mybir.ActivationFunctionType.Exp,
                        scale=0.125)
                    if kc == 0:
                        emit_finish_tail()
                    elif kc == 1:
                        emit_norm_tail()
                    pop_filler(j, qh, kc)
                    if kc > 0:
                        emit_pv(kc - 1)

                rcs = []

                def finish_tail(j=j, qh=qh, ps_pv=ps_pv, rcs=rcs,
                                emit_pv=emit_pv):
                    emit_pv(NKC - 1)
                    for hh in range(2):
                        rc_t = rc_pool.tile([1, 512], BF16, tag="rc",
                                            name=f"rc_{j}_{qh}_{hh}")
                        nc.vector.tensor_copy(rc_t, ps_pv[hh][D:DP, :])
                        rcs.append(rc_t)

                def norm_tail(j=j, qh=qh, q0=q0, ps_pv=ps_pv, rcs=rcs,
                              at_t=at_t):
                    bc_sb = bcast_pool.tile([64, 1024], F32, tag="bc",
                                            name=f"bc_{j}_{qh}")
                    bc_ps = ps.tile([64, 1024], F32, tag="scores", bufs=2,
                                    name=f"bcps_{j}_{qh}")
                    for hh in range(2):
                        nc.tensor.matmul(bc_ps[:, hh * 512:(hh + 1) * 512],
                                         ones_row, rcs[hh],
                                         start=True, stop=True)
                    nc.vector.reciprocal_approx_fast(out=bc_sb, in_=bc_ps)
                    for hh in range(2):
                        nc.vector.tensor_mul(
                            at_t[hh * 64:(hh + 1) * 64, q0:q0 + 512],
                            ps_pv[hh][0:D, :],
                            bc_sb[:, hh * 512:(hh + 1) * 512])

                pending_finish[0] = finish_tail
                pending_norm[0] = norm_tail
            attnT.append(at_t)
        emit_norm_tail()
        for key in sorted(filler_schedule):
            for thunk in filler_schedule[key]:
                thunk()
        filler_schedule.clear()
        wst_pool.release()
        wq_pool.release()

        # ---- output projection + bias ----
        for st in range(NST):
            y_t = ypool.tile([128, E], F32, tag="y", name=f"y_{st}")
            for (c0, cn) in ((0, 512), (512, 256)):
                ps_y = ps.tile([128, 512], F32, tag="mm", bufs=2,
                               name=f"psy_{st}_{c0}")
                for ej in range(NE):
                    nc.tensor.matmul(
                        ps_y[:, 0:cn],
                        attnT[ej][:, st * 128:(st + 1) * 128],
                        wo[ej][:, c0:c0 + cn],
                        start=(ej == 0), stop=(ej == NE - 1))
                nc.vector.tensor_add(y_t[:, c0:c0 + cn], ps_y[:, 0:cn],
                                     bias_bc[:, c0:c0 + cn])
            nc.sync.dma_start(out=y_d[st * 128:(st + 1) * 128, :], in_=y_t)


_NC_CACHE = None


def _get_nc():
    global _NC_CACHE
    if _NC_CACHE is None:
        _NC_CACHE = _build()
    return _NC_CACHE


def kernel(x, w_qkv, w_out, b_out, _trace=False, **_run_kwargs):
    """Full-input MHA: x [8,1024,768] f32 -> y [8,1024,768] f32."""
    nc = _get_nc()
    x = np.ascontiguousarray(np.asarray(x, dtype=np.float32))
    w_qkv = np.ascontiguousarray(np.asarray(w_qkv, dtype=np.float32))
    w_out = np.ascontiguousarray(np.asarray(w_out, dtype=np.float32))
    b_out = np.ascontiguousarray(np.asarray(b_out, dtype=np.float32))
    in_maps = [
        {"x": x[i], "w_qkv": w_qkv, "w_out": w_out, "b_out": b_out}
        for i in range(NCORES)
    ]
    res = run_bass_kernel_spmd(nc, in_maps, core_ids=list(range(NCORES)),
                               trace=_trace, **_run_kwargs)
    y = np.stack([res.results[i]["y"] for i in range(NCORES)], axis=0)
    if _trace:
        return y, res
    return y


# revision 11
# speedup vs baseline: 1.4191x; 1.3177x over previous
"""Multi-head attention (B=8, S=1024, E=768, H=12, D=64) on 8 TRN2 NeuronCores.

Sharding: data-parallel over batch. Core i computes batch element i end to end;
weights are replicated. No collectives.

All matmul operands are bf16 (contraction-128 bf16 matmuls measure ~1.45
cycles/row on TRN2 vs f32r's ~1.7; contraction-64 runs at 1.0).  Weights are
DMA'd as f32 into staging tiles and cast to bf16 on DVE, overlapped with the
x-transpose prelude and the attention loop (cast thunks ride the PE filler
schedule; DVE-only so the ACT exp chain never blocks on weight DMA).

Attention is software-pipelined one half-pair ("slot") ahead: slot s's scores
+ exp (producer) are emitted while slot s-1's PV (consumer) accumulates, so
ACT banks exp work and the PE never waits on exp in the filler-free tail.

Normalization uses a ones-block: each head's v_pad slab is [128 keys, 64 v
dims | 64 ones columns], so the PV matmul emits the softmax denominator
already broadcast across 64 partitions (rows 64-127 of the [128,512] psum
out).  Normalize = fast reciprocal + multiply on DVE; no PE broadcast, no
denominator row copies.

PSUM budget (8 banks): scores 2x2 + pv 2 (single [128,1024] slot) + mm 2x1.
"""

import numpy as np

import concourse.bass as bass
import concourse.bacc as bacc
import concourse.tile as tile
from concourse import mybir
from concourse.bass_utils import run_bass_kernel_spmd
from concourse.bass_interp import get_hw_module
from concourse.masks import make_identity

F32 = mybir.dt.float32
BF16 = mybir.dt.bfloat16
U32 = mybir.dt.uint32

B, S, E = 8, 1024, 768
H, D = 12, 64
F = 3 * E                  # 2304
NCORES = 8
NPAIR = H // 2             # 6 head pairs
NKC = S // 128             # 8 key chunks
NST = S // 128             # 8 sequence tiles
NE = E // 128              # 6 embedding chunks
NSLOT = 2 * NPAIR          # 12 (pair, q-half) slots
VW = 128                   # per-head v_pad slab: 64 v dims + 64 ones cols

BF16_ONES = 0x3F803F80     # two packed bf16 1.0


def _build():
    nc = bacc.Bacc("TRN2", target_bir_lowering=False, debug=False,
                   num_devices=NCORES)

    x_d = nc.dram_tensor("x", [S, E], F32, kind="ExternalInput").ap()
    wqkv_d = nc.dram_tensor("w_qkv", [E, F], F32, kind="ExternalInput").ap()
    wout_d = nc.dram_tensor("w_out", [E, E], F32, kind="ExternalInput").ap()
    bout_d = nc.dram_tensor("b_out", [E], F32, kind="ExternalInput").ap()
    y_d = nc.dram_tensor("y", [S, E], F32, kind="ExternalOutput").ap()

    with tile.TileContext(nc) as tc:
        _emit(nc, tc, x_d, wqkv_d, wout_d, bout_d, y_d)

    nc.compile()
    nc.m = get_hw_module(nc.m)
    return nc


def _emit(nc, tc, x_d, wqkv_d, wout_d, bout_d, y_d):
    from contextlib import ExitStack
    ctx = ExitStack()
    with ctx:
        singles = ctx.enter_context(tc.tile_pool(name="singles", bufs=1))
        sb = ctx.enter_context(tc.tile_pool(name="sb", bufs=1))
        ps = ctx.enter_context(tc.tile_pool(name="ps", bufs=1, space="PSUM"))
        expst_pool = ctx.enter_context(tc.tile_pool(name="expst", bufs=11))
        bcast_pool = ctx.enter_context(tc.tile_pool(name="bcast", bufs=2))
        ypool = ctx.enter_context(tc.tile_pool(name="ypool", bufs=2))
        wpool = ctx.enter_context(tc.tile_pool(name="wpool", bufs=1))

        # ---- constants ----
        identity = singles.tile([128, 128], BF16)
        make_identity(nc, identity)
        bias_bc = singles.tile([128, E], F32)

        wq_pool = tc.alloc_tile_pool(name="wq_pool", bufs=1)
        wst_pool = tc.alloc_tile_pool(name="wst_pool", bufs=1)
        x_pool = tc.alloc_tile_pool(name="x_pool", bufs=1)

        # bf16 weights for QKV projection: wq[ei] holds rows [128*ei, +128)
        wq = [wq_pool.tile([128, F], BF16, name=f"wqkv_{ei}")
              for ei in range(NE)]

        # f32 staging for weight chunks (DMA f32 -> cast bf16 on DVE).
        def dma_w_group(ei, c0, cn, tag, bufs):
            st_t = wst_pool.tile([128, cn], F32, tag=tag, bufs=bufs,
                                 name=f"wst_{ei}_{c0}")
            nc.sync.dma_start(out=st_t,
                              in_=wqkv_d[ei * 128:(ei + 1) * 128, c0:c0 + cn])
            return st_t

        def cast_w_group(ei, c0, st_t, on_act=False):
            dst = wq[ei][:, c0:c0 + st_t.shape[1]]
            if on_act:
                nc.scalar.copy(dst, st_t)
            else:
                nc.vector.tensor_copy(dst, st_t)

        # ---- x -> bf16 -> PE transpose -> xT [E, S] bf16 ----
        xT = [wq_pool.tile([128, S], BF16, name=f"xT_{ei}")
              for ei in range(NE)]
        w_stage = {}
        for half in range(2):
            xb_sb = []
            for k in range(4):
                st = half * 4 + k
                x_t = x_pool.tile([128, E], F32, tag="x", bufs=4,
                                  name=f"x_{st}")
                # split each x tile across two DMA queues
                nc.sync.dma_start(out=x_t[:, 0:E // 2],
                                  in_=x_d[st * 128:(st + 1) * 128, 0:E // 2])
                nc.sync.dma_start(out=x_t[:, E // 2:E],
                                  in_=x_d[st * 128:(st + 1) * 128, E // 2:E])
                xb = x_pool.tile([128, E], BF16, tag="xb", bufs=4,
                                 name=f"xb_{st}")
                if k % 2 == 0:
                    nc.vector.tensor_copy(xb, x_t)
                else:
                    nc.scalar.copy(xb, x_t)
                xb_sb.append(xb)
            if half == 0:
                for ei in range(NE):
                    w_stage[(ei, 0)] = dma_w_group(ei, 0, 128, "wsts", 12)
                    w_stage[(ei, E)] = dma_w_group(ei, E, 128, "wsts", 12)
            else:
                for ei in range(NE):
                    w_stage[(ei, 2 * E)] = dma_w_group(ei, 2 * E, E,
                                                       "wstb", 9)
                for ei in range(NE):
                    w_stage[(ei, 128)] = dma_w_group(ei, 128, E - 128,
                                                     "wstb", 9)
                for ei in range(NE):
                    w_stage[(ei, E + 128)] = dma_w_group(ei, E + 128, E - 128,
                                                         "wstb", 9)
            for ei in range(NE):
                ps_xt = ps.tile([128, 512], BF16, tag="mm", bufs=2,
                                name=f"psxt_{ei}_{half}")
                for k in range(4):
                    nc.tensor.transpose(
                        ps_xt[:, k * 128:(k + 1) * 128],
                        xb_sb[k][:, ei * 128:(ei + 1) * 128],
                        identity)
                dst = xT[ei][:, half * 512:(half + 1) * 512]
                if ei % 2 == 0:
                    nc.vector.tensor_copy(dst, ps_xt)
                else:
                    nc.scalar.copy(dst, ps_xt)
            # after the half-0 transposes: cast Q0/K0 groups (needed by the
            # prelude qkt chunks below)
            if half == 0:
                for ei in range(NE):
                    cast_w_group(ei, 0, w_stage[(ei, 0)], on_act=(ei % 2 == 0))
                    cast_w_group(ei, E, w_stage[(ei, E)], on_act=(ei % 2 == 1))
        x_pool.release()

        # ---- projection chunk emitters (used as PE fillers) ----
        # v_pad[st]: per head a [128, 128] slab = 64 ones cols | 64 v dims
        # (ones first so the denominator lands at psum partition base 0,
        # where the custom-DVE reciprocal reads it).
        v_pad = [sb.tile([128, H * VW], BF16, name=f"vpad_{st}")
                 for st in range(NST)]
        for st in range(NST):
            nc.gpsimd.memset(v_pad[st].bitcast(U32), BF16_ONES)
        qkT = [sb.tile([128, S], BF16, name=f"qkT_{ft}")
               for ft in range(2 * NE)]

        def emit_v_chunk(st, c0, cn):
            ps_v = ps.tile([128, 512], F32, tag="mm", bufs=2,
                           name=f"psv_{st}_{c0}")
            for ei in range(NE):
                nc.tensor.matmul(
                    ps_v[:, 0:cn],
                    xT[ei][:, st * 128:(st + 1) * 128],
                    wq[ei][:, 2 * E + c0:2 * E + c0 + cn],
                    start=(ei == 0), stop=(ei == NE - 1))
            vp3 = v_pad[st].rearrange("p (h c) -> p h c", c=VW)
            h0 = c0 // D
            nc.vector.tensor_copy(
                vp3[:, h0:h0 + cn // D, D:VW],
                ps_v[:, 0:cn].rearrange("p (h d) -> p h d", d=D))

        def emit_qkt_chunk(ft, sc, on_act=False):
            ps_q = ps.tile([128, 512], F32, tag="mm", bufs=2,
                           name=f"psq_{ft}_{sc}")
            for ei in range(NE):
                nc.tensor.matmul(
                    ps_q,
                    wq[ei][:, ft * 128:(ft + 1) * 128],
                    xT[ei][:, sc * 512:(sc + 1) * 512],
                    start=(ei == 0), stop=(ei == NE - 1))
            dst = qkT[ft][:, sc * 512:(sc + 1) * 512]
            if on_act:
                nc.scalar.copy(dst, ps_q)
            else:
                nc.vector.tensor_copy(dst, ps_q)

        # prelude: the first producer slot needs q/k lower halves of pair 0
        emit_qkt_chunk(0, 0, on_act=True)
        emit_qkt_chunk(NE, 0, on_act=True)

        # V casts: DVE-only (ACT exp chain must never block on weight DMA)
        for ei in range(NE):
            cast_w_group(ei, 2 * E, w_stage[(ei, 2 * E)], on_act=False)

        # ---- w_out staging (DMA early, cast via fillers) ----
        wo = [wpool.tile([128, E], BF16, name=f"wout_{ei}")
              for ei in range(NE)]
        wo_stage = {}
        for ei in range(NE):
            st_t = wst_pool.tile([128, E], F32, tag="wstb", bufs=9,
                                 name=f"wost_{ei}")
            nc.sync.dma_start(
                out=st_t, in_=wout_d[ei * 128:(ei + 1) * 128, :])
            wo_stage[ei] = st_t
        nc.sync.dma_start(
            out=bias_bc,
            in_=bass.AP(tensor=bout_d.tensor, offset=bout_d.offset,
                        ap=[[0, 128]] + list(bout_d.ap)))

        # ---- filler schedule: (slot, kc) -> thunks ----
        filler_schedule = {}

        def sched(s, kc, thunk):
            filler_schedule.setdefault((s, kc), []).append(thunk)

        def pop_filler(s, kc):
            for thunk in filler_schedule.pop((s, kc), ()):
                thunk()

        # weight casts (DVE): Qrest needed by qkt(1,0) at (0,3); Krest by
        # qkt(7,0) at (0,6)
        for ei in range(NE):
            sched(0, ei // 2,
                  lambda ei=ei: cast_w_group(ei, 128, w_stage[(ei, 128)]))
        for ei in range(NE):
            sched(0, 3 + ei // 2,
                  lambda ei=ei: cast_w_group(ei, E + 128,
                                             w_stage[(ei, E + 128)]))
        for ei in range(NE):
            sched(4 + ei % 2, 1 + 2 * (ei // 2),
                  lambda ei=ei: nc.vector.tensor_copy(wo[ei], wo_stage[ei]))

        # upper q/k halves of pair 0: needed by producer slot 1
        sched(0, 0, lambda: emit_qkt_chunk(0, 1))
        sched(0, 0, lambda: emit_qkt_chunk(NE, 1))
        # V chunks: v_pad[k] needed by PV(slot 0, k) during loop iter 1 at
        # kc=k+1 (PV(0,7) at iter 2 kc0)
        for st in range(NST):
            if st < 4:
                s, kc = 0, 1 + st
            else:
                s, kc = 1, st - 4
            sched(s, kc, lambda st=st: emit_v_chunk(st, 0, 512))
            sched(s, kc, lambda st=st: emit_v_chunk(st, 512, 256))
        # qkt chunks for pair j: producer slot 2j needs (j,0) and (NE+j,0);
        # kT cols 512+ from kc4; qT upper half at slot 2j+1
        for j in range(1, NPAIR):
            if j == 1:
                sched(0, 3, lambda: emit_qkt_chunk(1, 0))
                sched(0, 6, lambda: emit_qkt_chunk(NE + 1, 0))
                sched(1, 4, lambda: emit_qkt_chunk(NE + 1, 1))
                sched(1, 6, lambda: emit_qkt_chunk(1, 1))
            else:
                sched(2 * j - 2, 3, lambda j=j: emit_qkt_chunk(j, 0))
                sched(2 * j - 2, 6, lambda j=j: emit_qkt_chunk(NE + j, 0))
                sched(2 * j - 1, 4, lambda j=j: emit_qkt_chunk(NE + j, 1))
                sched(2 * j - 1, 6, lambda j=j: emit_qkt_chunk(j, 1))

        # ---- attention: software-pipelined producer/consumer slots ----
        attnT = [sb.tile([128, S], BF16, name=f"attnT_{j}")
                 for j in range(NPAIR)]
        expst_tiles = {}
        ps_pv_of = {}
        pending_finish = [None]
        pending_norm = [None]

        def emit_finish_tail():
            if pending_finish[0] is not None:
                pending_finish[0]()
                pending_finish[0] = None

        def emit_norm_tail():
            emit_finish_tail()
            if pending_norm[0] is not None:
                pending_norm[0]()
                pending_norm[0] = None

        def emit_pv(c, kc):
            j, qh = c // 2, c % 2
            ps_pv = ps_pv_of[c]
            expst = expst_tiles.pop((c, kc))
            for hh in range(2):
                nc.tensor.matmul(
                    ps_pv[:, hh * 512:(hh + 1) * 512],
                    v_pad[kc][:, (2 * j + hh) * VW:(2 * j + hh + 1) * VW],
                    expst[:, hh * 512:(hh + 1) * 512],
                    start=(kc == 0), stop=(kc == NKC - 1))

        def finish_tail(c):
            emit_pv(c, NKC - 1)

        def norm_tail(c):
            j, qh = c // 2, c % 2
            q0 = qh * 512
            ps_pv = ps_pv_of.pop(c)
            bc = bcast_pool.tile([64, 1024], F32, tag="bc", name=f"bc_{c}")
            nc.vector.reciprocal_approx_fast(out=bc, in_=ps_pv[0:64, :])
            for hh in range(2):
                nc.vector.tensor_mul(
                    attnT[j][hh * 64:(hh + 1) * 64, q0:q0 + 512],
                    ps_pv[64:128, hh * 512:(hh + 1) * 512],
                    bc[:, hh * 512:(hh + 1) * 512])

        for s in range(NSLOT + 1):
            if s >= 1:
                # consumer c = s-1 accumulates PV(kc 0..6) this iteration
                ps_pv_of[s - 1] = ps.tile([128, 1024], F32, tag="pv", bufs=1,
                                          name=f"pspv_{s - 1}")
            for kc in range(NKC):
                if s < NSLOT:
                    j, qh = s // 2, s % 2
                    q0 = qh * 512
                    qT = qkT[j]
                    kT = qkT[NE + j]
                    expst = expst_pool.tile([128, 1024], BF16, tag="expst",
                                            name=f"expst_{s}_{kc}")
                    expst_tiles[(s, kc)] = expst
                    ps_s = ps.tile([128, 1024], F32, tag="scores", bufs=2,
                                   name=f"pss_{s}_{kc}")
                    for hh in range(2):
                        nc.tensor.matmul(
                            ps_s[:, hh * 512:(hh + 1) * 512],
                            kT[hh * 64:(hh + 1) * 64,
                               kc * 128:(kc + 1) * 128],
                            qT[hh * 64:(hh + 1) * 64, q0:q0 + 512],
                            start=True, stop=True,
                            tile_position=(hh * 64, 0))
                    nc.scalar.activation(
                        out=expst, in_=ps_s,
                        func=mybir.ActivationFunctionType.Exp,
                        scale=0.125)
                if kc == 0:
                    emit_finish_tail()
                elif kc == 1:
                    emit_norm_tail()
                pop_filler(s, kc)
                if s >= 1 and kc >= 1:
                    emit_pv(s - 1, kc - 1)
            if s >= 1:
                c = s - 1
                pending_finish[0] = lambda c=c: finish_tail(c)
                pending_norm[0] = lambda c=c: norm_tail(c)
        emit_norm_tail()
        for key in sorted(filler_schedule):
            for thunk in filler_schedule[key]:
                thunk()
        filler_schedule.clear()
        wst_pool.release()
        wq_pool.release()

        # ---- output projection + bias ----
        for st in range(NST):
            y_t = ypool.tile([128, E], F32, tag="y", name=f"y_{st}")
            for (c0, cn) in ((0, 512), (512, 256)):
                ps_y = ps.tile([128, 512], F32, tag="mm", bufs=2,
                               name=f"psy_{st}_{c0}")
                for ej in range(NE):
                    nc.tensor.matmul(
                        ps_y[:, 0:cn],
                        attnT[ej][:, st * 128:(st + 1) * 128],
                        wo[ej][:, c0:c0 + cn],
                        start=(ej == 0), stop=(ej == NE - 1))
                nc.vector.tensor_add(y_t[:, c0:c0 + cn], ps_y[:, 0:cn],
                                     bias_bc[:, c0:c0 + cn])
            nc.sync.dma_start(out=y_d[st * 128:(st + 1) * 128, :], in_=y_t)


_NC_CACHE = None


def _get_nc():
    global _NC_CACHE
    if _NC_CACHE is None:
        _NC_CACHE = _build()
    return _NC_CACHE


def kernel(x, w_qkv, w_out, b_out, _trace=False, **_run_kwargs):
    """Full-input MHA: x [8,1024,768] f32 -> y [8,1024,768] f32."""
    nc = _get_nc()
    x = np.ascontiguousarray(np.asarray(x, dtype=np.float32))
    w_qkv = np.ascontiguousarray(np.asarray(w_qkv, dtype=np.float32))
    w_out = np.ascontiguousarray(np.asarray(w_out, dtype=np.float32))
    b_out = np.ascontiguousarray(np.asarray(b_out, dtype=np.float32))
    in_maps = [
        {"x": x[i], "w_qkv": w_qkv, "w_out": w_out, "b_out": b_out}
        for i in range(NCORES)
    ]
    res = run_bass_kernel_spmd(nc, in_maps, core_ids=list(range(NCORES)),
                               trace=_trace, **_run_kwargs)
    y = np.stack([res.results[i]["y"] for i in range(NCORES)], axis=0)
    if _trace:
        return y, res
    return y


# revision 12
# speedup vs baseline: 1.4271x; 1.0056x over previous
"""Multi-head attention (B=8, S=1024, E=768, H=12, D=64) on 8 TRN2 NeuronCores.

Sharding: data-parallel over batch. Core i computes batch element i end to end;
weights are replicated. No collectives.

All matmul operands are bf16 (contraction-128 bf16 matmuls stream noticeably
faster than f32r on TRN2; dense PE work also holds the clock at boost).
Weights are DMA'd as f32 into staging tiles and cast to bf16, overlapped with
the x-transpose prelude and the attention loop (cast thunks ride the PE filler
schedule; DVE-only during attention so the ACT exp chain never blocks on
weight DMA).

Attention runs as a flat software pipeline over 96 global chunks (slot s =
(pair, q-half), kc = 128-key block; chunk g = 8s+kc).  The producer emits
scores (PE) + exp (ACT) for chunk g while the consumer accumulates PV for
chunk g-LEAD, so ACT banks LEAD chunks of exp work in an SBUF ring during the
PE-heavy projection phase and the PE never waits on exp in the filler-free
tail.  The last LEAD steps are consume-only (pure PE work).

Normalization uses a ones-block: each head's v_pad slab is [128 keys, 64 ones
cols | 64 v dims], so the PV matmul emits the softmax denominator already
broadcast across psum rows 0-63 (partition base 0, where the custom-DVE
reciprocal requires its input).  Normalize = fast reciprocal + two multiplies
on DVE; no PE broadcast, no denominator row copies.

PSUM budget (8 banks): scores 2x2 + pv 2 (single [128,1024] slot) + mm 2x1.
"""

import numpy as np

import concourse.bass as bass
import concourse.bacc as bacc
import concourse.tile as tile
from concourse import mybir
from concourse.bass_utils import run_bass_kernel_spmd
from concourse.bass_interp import get_hw_module
from concourse.masks import make_identity

F32 = mybir.dt.float32
BF16 = mybir.dt.bfloat16
U32 = mybir.dt.uint32

B, S, E = 8, 1024, 768
H, D = 12, 64
F = 3 * E                  # 2304
NCORES = 8
NPAIR = H // 2             # 6 head pairs
NKC = S // 128             # 8 key chunks
NST = S // 128             # 8 sequence tiles
NE = E // 128              # 6 embedding chunks
NSLOT = 2 * NPAIR          # 12 (pair, q-half) slots
VW = 128                   # per-head v_pad slab: 64 ones cols + 64 v dims
LEAD = 12                  # producer-consumer distance in chunks
NCH = NSLOT * NKC          # 96 chunks

BF16_ONES = 0x3F803F80     # two packed bf16 1.0


def _build():
    nc = bacc.Bacc("TRN2", target_bir_lowering=False, debug=False,
                   num_devices=NCORES)

    x_d = nc.dram_tensor("x", [S, E], F32, kind="ExternalInput").ap()
    wqkv_d = nc.dram_tensor("w_qkv", [E, F], F32, kind="ExternalInput").ap()
    wout_d = nc.dram_tensor("w_out", [E, E], F32, kind="ExternalInput").ap()
    bout_d = nc.dram_tensor("b_out", [E], F32, kind="ExternalInput").ap()
    y_d = nc.dram_tensor("y", [S, E], F32, kind="ExternalOutput").ap()

    with tile.TileContext(nc) as tc:
        _emit(nc, tc, x_d, wqkv_d, wout_d, bout_d, y_d)

    nc.compile()
    nc.m = get_hw_module(nc.m)
    return nc


def _emit(nc, tc, x_d, wqkv_d, wout_d, bout_d, y_d):
    from contextlib import ExitStack
    ctx = ExitStack()
    with ctx:
        singles = ctx.enter_context(tc.tile_pool(name="singles", bufs=1))
        sb = ctx.enter_context(tc.tile_pool(name="sb", bufs=1))
        ps = ctx.enter_context(tc.tile_pool(name="ps", bufs=1, space="PSUM"))
        bcast_pool = ctx.enter_context(tc.tile_pool(name="bcast", bufs=2))
        ypool = ctx.enter_context(tc.tile_pool(name="ypool", bufs=2))
        wpool = ctx.enter_context(tc.tile_pool(name="wpool", bufs=1))

        # ---- constants ----
        identity = singles.tile([128, 128], BF16)
        make_identity(nc, identity)
        bias_bc = singles.tile([128, E], F32)

        wq_pool = tc.alloc_tile_pool(name="wq_pool", bufs=1)
        wst_pool = tc.alloc_tile_pool(name="wst_pool", bufs=1)
        x_pool = tc.alloc_tile_pool(name="x_pool", bufs=1)

        # bf16 weights for QKV projection: wq[ei] holds rows [128*ei, +128)
        wq = [wq_pool.tile([128, F], BF16, name=f"wqkv_{ei}")
              for ei in range(NE)]

        # f32 staging for weight chunks (DMA f32 -> cast bf16).
        def dma_w_group(ei, c0, cn, tag, bufs):
            st_t = wst_pool.tile([128, cn], F32, tag=tag, bufs=bufs,
                                 name=f"wst_{ei}_{c0}")
            nc.sync.dma_start(out=st_t,
                              in_=wqkv_d[ei * 128:(ei + 1) * 128, c0:c0 + cn])
            return st_t

        def cast_w_group(ei, c0, st_t, on_act=False):
            dst = wq[ei][:, c0:c0 + st_t.shape[1]]
            if on_act:
                nc.scalar.copy(dst, st_t)
            else:
                nc.vector.tensor_copy(dst, st_t)

        # v_pad[st]: per head a [128, 128] slab = 64 ones cols | 64 v dims
        # (ones first so the denominator lands at psum partition base 0,
        # where the custom-DVE reciprocal reads it).
        v_pad = [sb.tile([128, H * VW], BF16, name=f"vpad_{st}")
                 for st in range(NST)]
        for st in range(NST):
            nc.gpsimd.memset(v_pad[st].bitcast(U32), BF16_ONES)
        qkT = [sb.tile([128, S], BF16, name=f"qkT_{ft}")
               for ft in range(2 * NE)]

        # ---- x -> bf16 -> PE transpose -> xT [E, S] bf16 ----
        # Half 0 also accumulates the pair-0 qkt chunks (ft 0 and NE) per ei
        # right after each xT drain, so the first scores/exp start early.
        xT = [wq_pool.tile([128, S], BF16, name=f"xT_{ei}")
              for ei in range(NE)]
        w_stage = {}
        ps_qkt = {}
        for half in range(2):
            x_tiles = []
            for k in range(4):
                st = half * 4 + k
                x_t = x_pool.tile([128, E], F32, tag="x", bufs=4,
                                  name=f"x_{st}")
                for q in range(3):
                    nc.sync.dma_start(
                        out=x_t[:, q * 256:(q + 1) * 256],
                        in_=x_d[st * 128:(st + 1) * 128,
                                q * 256:(q + 1) * 256])
                x_tiles.append((st, x_t))
            if half == 0:
                for ei in range(NE):
                    w_stage[(ei, 0)] = dma_w_group(ei, 0, 128, "wsts", 12)
                    w_stage[(ei, E)] = dma_w_group(ei, E, 128, "wsts", 12)
            else:
                for ei in range(NE):
                    w_stage[(ei, 2 * E)] = dma_w_group(ei, 2 * E, E,
                                                       "wstb", 7)
                for ei in range(NE):
                    w_stage[(ei, 128)] = dma_w_group(ei, 128, E - 128,
                                                     "wstb", 7)
                for ei in range(NE):
                    w_stage[(ei, E + 128)] = dma_w_group(ei, E + 128, E - 128,
                                                         "wstb", 7)
            # cast x -> bf16 per 256-col chunk (transposes for ei pair q can
            # start as soon as chunk q of all four tiles landed)
            xbb = []
            for (st, x_t) in x_tiles:
                xb = x_pool.tile([128, E], BF16, tag="xb", bufs=4,
                                 name=f"xb_{st}")
                for q in range(3):
                    src = x_t[:, q * 256:(q + 1) * 256]
                    dst = xb[:, q * 256:(q + 1) * 256]
                    if (st + q) % 2 == 0:
                        nc.vector.tensor_copy(dst, src)
                    else:
                        nc.scalar.copy(dst, src)
                xbb.append(xb)
            if half == 0:
                for ei in range(NE):
                    cast_w_group(ei, 0, w_stage[(ei, 0)],
                                 on_act=(ei % 2 == 0))
                    cast_w_group(ei, E, w_stage[(ei, E)],
                                 on_act=(ei % 2 == 1))
                for ft in (0, NE):
                    ps_qkt[ft] = ps.tile([128, 512], F32, tag="scores",
                                         bufs=2, name=f"psqkt_{ft}")
            for ei in range(NE):
                ps_xt = ps.tile([128, 512], BF16, tag="mm", bufs=2,
                                name=f"psxt_{ei}_{half}")
                for k in range(4):
                    nc.tensor.transpose(
                        ps_xt[:, k * 128:(k + 1) * 128],
                        xbb[k][:, ei * 128:(ei + 1) * 128],
                        identity)
                dst = xT[ei][:, half * 512:(half + 1) * 512]
                if ei % 2 == 0:
                    nc.vector.tensor_copy(dst, ps_xt)
                else:
                    nc.scalar.copy(dst, ps_xt)
                if half == 0:
                    for ft in (0, NE):
                        nc.tensor.matmul(
                            ps_qkt[ft],
                            wq[ei][:, ft * 128:(ft + 1) * 128],
                            xT[ei][:, 0:512],
                            start=(ei == 0), stop=(ei == NE - 1))
            if half == 0:
                nc.scalar.copy(qkT[0][:, 0:512], ps_qkt[0])
                nc.vector.tensor_copy(qkT[NE][:, 0:512], ps_qkt[NE])
        x_pool.release()

        # expst ring reuses the released x staging space (opened after
        # x_pool.release(); released before wst/wq below)
        expst_pool = tc.alloc_tile_pool(name="expst_pool", bufs=15)

        # ---- projection chunk emitters (PE fillers) ----
        def emit_v_chunk(st, c0, cn):
            ps_v = ps.tile([128, 512], F32, tag="mm", bufs=2,
                           name=f"psv_{st}_{c0}")
            for ei in range(NE):
                nc.tensor.matmul(
                    ps_v[:, 0:cn],
                    xT[ei][:, st * 128:(st + 1) * 128],
                    wq[ei][:, 2 * E + c0:2 * E + c0 + cn],
                    start=(ei == 0), stop=(ei == NE - 1))
            vp3 = v_pad[st].rearrange("p (h c) -> p h c", c=VW)
            h0 = c0 // D
            nc.vector.tensor_copy(
                vp3[:, h0:h0 + cn // D, D:VW],
                ps_v[:, 0:cn].rearrange("p (h d) -> p h d", d=D))

        def emit_qkt_chunk(ft, sc):
            ps_q = ps.tile([128, 512], F32, tag="mm", bufs=2,
                           name=f"psq_{ft}_{sc}")
            for ei in range(NE):
                nc.tensor.matmul(
                    ps_q,
                    wq[ei][:, ft * 128:(ft + 1) * 128],
                    xT[ei][:, sc * 512:(sc + 1) * 512],
                    start=(ei == 0), stop=(ei == NE - 1))
            nc.vector.tensor_copy(qkT[ft][:, sc * 512:(sc + 1) * 512], ps_q)

        # V casts: DVE-only (ACT exp chain must never block on weight DMA)
        for ei in range(NE):
            cast_w_group(ei, 2 * E, w_stage[(ei, 2 * E)], on_act=False)

        # ---- w_out staging (DMA early, cast via fillers) ----
        wo = [wpool.tile([128, E], BF16, name=f"wout_{ei}")
              for ei in range(NE)]
        wo_stage = {}
        for ei in range(NE):
            st_t = wst_pool.tile([128, E], F32, tag="wstb", bufs=7,
                                 name=f"wost_{ei}")
            nc.sync.dma_start(
                out=st_t, in_=wout_d[ei * 128:(ei + 1) * 128, :])
            wo_stage[ei] = st_t
        nc.sync.dma_start(
            out=bias_bc,
            in_=bass.AP(tensor=bout_d.tensor, offset=bout_d.offset,
                        ap=[[0, 128]] + list(bout_d.ap)))

        # ---- filler schedule: producer step (8*slot + kc) -> thunks ----
        filler_schedule = {}

        def sched(step, thunk):
            filler_schedule.setdefault(step, []).append(thunk)

        def pop_filler(step):
            for thunk in filler_schedule.pop(step, ()):
                thunk()

        # weight casts (DVE)
        for ei in range(NE):
            sched(ei // 2,
                  lambda ei=ei: cast_w_group(ei, 128, w_stage[(ei, 128)]))
        for ei in range(NE):
            sched(3 + ei // 2,
                  lambda ei=ei: cast_w_group(ei, E + 128,
                                             w_stage[(ei, E + 128)]))
        for ei in range(NE):
            sched(33 + 2 * ei,
                  lambda ei=ei: nc.vector.tensor_copy(wo[ei], wo_stage[ei]))

        # upper q/k halves of pair 0
        sched(0, lambda: emit_qkt_chunk(0, 1))
        sched(0, lambda: emit_qkt_chunk(NE, 1))
        # V chunks: v_pad[k] consumed by PV chunk k at step k+LEAD
        for st in range(NST):
            step = 1 + st if st < 4 else 4 + st
            sched(step, lambda st=st: emit_v_chunk(st, 0, 512))
            sched(step, lambda st=st: emit_v_chunk(st, 512, 256))
        # qkt chunks for pair j: (j,0) & (NE+j,0) by step 16j; (NE+j,1) by
        # 16j+4; (j,1) by 16j+8
        for j in range(1, NPAIR):
            base = 16 * (j - 1)
            sched(base + 3, lambda j=j: emit_qkt_chunk(j, 0))
            sched(base + 6, lambda j=j: emit_qkt_chunk(NE + j, 0))
            sched(base + 12, lambda j=j: emit_qkt_chunk(NE + j, 1))
            sched(base + 14, lambda j=j: emit_qkt_chunk(j, 1))

        # ---- attention: flat-step pipelined producer/consumer ----
        attnT = [sb.tile([128, S], BF16, name=f"attnT_{j}")
                 for j in range(NPAIR)]
        expst_tiles = {}
        ps_pv_of = {}

        def norm_tail(c):
            j, qh = c // 2, c % 2
            q0 = qh * 512
            ps_pv = ps_pv_of.pop(c)
            bc = bcast_pool.tile([64, 1024], F32, tag="bc", name=f"bc_{c}")
            nc.vector.reciprocal_approx_fast(out=bc, in_=ps_pv[0:64, :])
            for hh in range(2):
                nc.vector.tensor_mul(
                    attnT[j][hh * 64:(hh + 1) * 64, q0:q0 + 512],
                    ps_pv[64:128, hh * 512:(hh + 1) * 512],
                    bc[:, hh * 512:(hh + 1) * 512])

        for g in range(NCH + LEAD):
            if g < NCH:
                s, kc = divmod(g, NKC)
                j, qh = s // 2, s % 2
                q0 = qh * 512
                qT = qkT[j]
                kT = qkT[NE + j]
                expst = expst_pool.tile([128, 1024], BF16, tag="expst",
                                        name=f"expst_{g}")
                expst_tiles[g] = expst
                ps_s = ps.tile([128, 1024], F32, tag="scores", bufs=2,
                               name=f"pss_{g}")
                for hh in range(2):
                    nc.tensor.matmul(
                        ps_s[:, hh * 512:(hh + 1) * 512],
                        kT[hh * 64:(hh + 1) * 64, kc * 128:(kc + 1) * 128],
                        qT[hh * 64:(hh + 1) * 64, q0:q0 + 512],
                        start=True, stop=True,
                        tile_position=(hh * 64, 0))
                nc.scalar.activation(
                    out=expst, in_=ps_s,
                    func=mybir.ActivationFunctionType.Exp,
                    scale=0.125)
            pop_filler(g)
            cg = g - LEAD
            if cg >= 0:
                c, ckc = divmod(cg, NKC)
                cj = c // 2
                if ckc == 0:
                    ps_pv_of[c] = ps.tile([128, 1024], F32, tag="pv",
                                          bufs=1, name=f"pspv_{c}")
                ps_pv = ps_pv_of[c]
                cexp = expst_tiles.pop(cg)
                for hh in range(2):
                    nc.tensor.matmul(
                        ps_pv[:, hh * 512:(hh + 1) * 512],
                        v_pad[ckc][:, (2 * cj + hh) * VW:
                                   (2 * cj + hh + 1) * VW],
                        cexp[:, hh * 512:(hh + 1) * 512],
                        start=(ckc == 0), stop=(ckc == NKC - 1))
                if ckc == NKC - 1:
                    norm_tail(c)
        for key in sorted(filler_schedule):
            for thunk in filler_schedule[key]:
                thunk()
        filler_schedule.clear()
        expst_pool.release()
        wst_pool.release()
        wq_pool.release()

        # ---- output projection + bias ----
        for st in range(NST):
            y_t = ypool.tile([128, E], F32, tag="y", name=f"y_{st}")
            for (c0, cn) in ((0, 512), (512, 256)):
                ps_y = ps.tile([128, 512], F32, tag="mm", bufs=2,
                               name=f"psy_{st}_{c0}")
                for ej in range(NE):
                    nc.tensor.matmul(
                        ps_y[:, 0:cn],
                        attnT[ej][:, st * 128:(st + 1) * 128],
                        wo[ej][:, c0:c0 + cn],
                        start=(ej == 0), stop=(ej == NE - 1))
                nc.vector.tensor_add(y_t[:, c0:c0 + cn], ps_y[:, 0:cn],
                                     bias_bc[:, c0:c0 + cn])
            nc.sync.dma_start(out=y_d[st * 128:(st + 1) * 128, :], in_=y_t)


_NC_CACHE = None


def _get_nc():
    global _NC_CACHE
    if _NC_CACHE is None:
        _NC_CACHE = _build()
    return _NC_CACHE


def kernel(x, w_qkv, w_out, b_out, _trace=False, **_run_kwargs):
    """Full-input MHA: x [8,1024,768] f32 -> y [8,1024,768] f32."""
    nc = _get_nc()
    x = np.ascontiguousarray(np.asarray(x, dtype=np.float32))
    w_qkv = np.ascontiguousarray(np.asarray(w_qkv, dtype=np.float32))
    w_out = np.ascontiguousarray(np.asarray(w_out, dtype=np.float32))
    b_out = np.ascontiguousarray(np.asarray(b_out, dtype=np.float32))
    in_maps = [
        {"x": x[i], "w_qkv": w_qkv, "w_out": w_out, "b_out": b_out}
        for i in range(NCORES)
    ]
    res = run_bass_kernel_spmd(nc, in_maps, core_ids=list(range(NCORES)),
                               trace=_trace, **_run_kwargs)
    y = np.stack([res.results[i]["y"] for i in range(NCORES)], axis=0)
    if _trace:
        return y, res
    return y
